# revision 18
# baseline (speedup 1.0000x reference)
"""L1 loss (mean |yhat - y|) over (64, 128, 4096) fp32 tensors on 8 TRN2 cores.

Strategy: pure data-parallel over the batch dim; core i takes batch rows
[8i, 8i+8), viewed as a [128, 32768] matrix (partition-major). The host
quantizes the inputs (rel-err budget is 2e-2; measured end-to-end error
~5e-4) and splits the columns into a staircase of tiles, each stored as
one interleaved [128, 2*cols] DRAM tensor ([yhat | y]) so a tile loads
with a single DMA. All tile DMAs are issued up front (the working set
fits in SBUF), and the stream saturates the per-core HBM share
(~350 GB/s under 8-core contention).

Three streams are balanced against each other (measured rates):
 - DMA: fp8 cols cost 1 B/elem, bf16 cols 2 B/elem -> ~10.8 MiB total.
 - Vector engine: per-tile d = yhat - y. fp8 inputs run tensor_tensor
   at 1x (1.04 ns/col), bf16 at 2x_1p (0.52 ns/col); the fp8/bf16 col
   split makes DVE finish with the DMA stream. One late tile's reduce
   also runs here (tensor_reduce with apply_absolute_value).
 - Scalar (ACT) engine: fused abs+sum via activation(Abs, accum_out=)
   at 0.83 ns/col, dtype-independent.
The tile staircase (small tiles first and last) starts both compute
engines ~5us earlier and shortens the post-DMA drain; warmup/cooldown
sizes came from an event-driven schedule simulation calibrated on
perfetto traces. (Fast DVE reduce paths don't exist: tensor_reduce is
1x by spec and tensor_scalar/scalar_tensor_tensor drop to 1x when an
accumulator output is attached; SWDGE accum-DMA subtraction runs ~4x
below line rate; so sub-on-DVE + reduce-on-ACT is the optimal split.)

Partials land in fp32 columns of a [128, n_tiles] accumulator; the
host combines them in float64 and divides by the global element count.
"""

import numpy as np
import ml_dtypes

import concourse.bacc as bacc
import concourse.mybir as mybir
import concourse.tile as tile
from concourse.bass_utils import run_bass_kernel_spmd

N_CORES = 8
FULL_SHAPE = (64, 128, 4096)
TOTAL_ELEMS = FULL_SHAPE[0] * FULL_SHAPE[1] * FULL_SHAPE[2]  # 33,554,432

P = 128                                   # SBUF partitions
COLS_PER_CORE = TOTAL_ELEMS // N_CORES // P  # 32,768 cols per input tensor

# (dtype, cols, reduce engine) per tile; cols sum to COLS_PER_CORE.
TILES = [
    ("f8", 1024, "act"),
    ("f8", 2048, "act"),
    ("f8", 4096, "act"),
    ("f8", 3072, "act"),
    ("b16", 4096, "act"),
    ("f8", 4096, "act"),
    ("f8", 2048, "act"),
    ("b16", 1024, "act"),
    ("f8", 2048, "act"),
    ("b16", 4096, "act"),
    ("b16", 3072, "act"),
    ("b16", 2048, "dve"),
]
assert sum(c for _, c, _ in TILES) == COLS_PER_CORE

_MDT = {"f8": mybir.dt.float8e4, "b16": mybir.dt.bfloat16}
_NDT = {"f8": ml_dtypes.float8_e4m3, "b16": ml_dtypes.bfloat16}

_nc_cache = []


def _build_nc():
    nc = bacc.Bacc("TRN2", target_bir_lowering=False, debug=False)
    zs = [
        nc.declare_dram_parameter(f"z{i}", [P, 2 * c], _MDT[dt], isOutput=False)
        for i, (dt, c, _) in enumerate(TILES)
    ]
    n = len(TILES)
    out = nc.declare_dram_parameter("out", [P, n], mybir.dt.float32, isOutput=True)

    with tile.TileContext(nc) as tc:
        with (
            tc.tile_pool(name="io", bufs=1) as io_pool,
            tc.tile_pool(name="wk", bufs=2) as wk_pool,
            tc.tile_pool(name="acc", bufs=1) as acc_pool,
        ):
            acc = acc_pool.tile([P, n], mybir.dt.float32)
            zts = []
            for i, (dt, c, _) in enumerate(TILES):
                zt = io_pool.tile([P, 2 * c], _MDT[dt], tag=f"z{i}")
                nc.sync.dma_start(zt[:], zs[i][:])
                zts.append(zt)
            for i, (dt, c, eng) in enumerate(TILES):
                d = wk_pool.tile([P, c], mybir.dt.bfloat16, tag="d")
                nc.vector.tensor_sub(d[:], zts[i][:, 0:c], zts[i][:, c : 2 * c])
                if eng == "act":
                    a = wk_pool.tile([P, c], mybir.dt.bfloat16, tag="a")
                    nc.scalar.activation(
                        a[:],
                        d[:],
                        mybir.ActivationFunctionType.Abs,
                        accum_out=acc[:, i : i + 1],
                    )
                else:
                    nc.vector.tensor_reduce(
                        acc[:, i : i + 1],
                        d[:],
                        axis=mybir.AxisListType.X,
                        op=mybir.AluOpType.add,
                        apply_absolute_value=True,
                    )
            nc.sync.dma_start(out[:], acc[:])
    nc.compile()
    return nc


def _get_nc():
    if not _nc_cache:
        _nc_cache.append(_build_nc())
    return _nc_cache[0]


def _shard_inputs(yhat: np.ndarray, y: np.ndarray) -> list[dict[str, np.ndarray]]:
    yhat_m = np.ascontiguousarray(yhat, dtype=np.float32).reshape(
        N_CORES, P, COLS_PER_CORE
    )
    y_m = np.ascontiguousarray(y, dtype=np.float32).reshape(
        N_CORES, P, COLS_PER_CORE
    )
    in_maps = [{} for _ in range(N_CORES)]
    off = 0
    for i, (dt, c, _) in enumerate(TILES):
        zt = np.empty((N_CORES, P, 2 * c), dtype=_NDT[dt])
        zt[:, :, 0:c] = yhat_m[:, :, off : off + c]
        zt[:, :, c : 2 * c] = y_m[:, :, off : off + c]
        for core in range(N_CORES):
            in_maps[core][f"z{i}"] = zt[core]
        off += c
    return in_maps


def kernel(yhat: np.ndarray, y: np.ndarray) -> np.ndarray:
    nc = _get_nc()
    in_maps = _shard_inputs(yhat, y)
    res = run_bass_kernel_spmd(nc, in_maps, list(range(N_CORES)))
    total = np.float64(0.0)
    for r in res.results:
        total += r["out"].astype(np.float64).sum()
    return np.asarray(total / TOTAL_ELEMS, dtype=np.float32)


# revision 19
# speedup vs baseline: 1.0152x; 1.0152x over previous
"""L1 loss (mean |yhat - y|) over (64, 128, 4096) fp32 tensors on 8 TRN2 cores.

Strategy: pure data-parallel over the batch dim; core i takes batch rows
[8i, 8i+8), viewed as a [128, 32768] matrix (partition-major). The host
quantizes the inputs (rel-err budget is 2e-2; measured end-to-end error
~5e-4) and splits the columns into a staircase of tiles, each stored as
one interleaved [128, 2*cols] DRAM tensor ([yhat | y]) so a tile loads
with a single DMA. All tile DMAs are issued up front (the working set
fits in SBUF), and the stream saturates the per-core HBM share
(~350 GB/s under 8-core contention).

Three streams are balanced against each other (measured rates):
 - DMA: fp8 cols cost 1 B/elem, bf16 cols 2 B/elem -> ~10.8 MiB total.
 - Vector engine: per-tile d = yhat - y. fp8 inputs run tensor_tensor
   at 1x (1.04 ns/col), bf16 at 2x_1p (0.52 ns/col); the fp8/bf16 col
   split makes DVE finish with the DMA stream. One late tile's reduce
   also runs here (tensor_reduce with apply_absolute_value).
 - Scalar (ACT) engine: fused abs+sum via activation(Abs, accum_out=)
   at 0.83 ns/col, dtype-independent.
The tile staircase (small tiles first and last) starts both compute
engines ~5us earlier and shortens the post-DMA drain; warmup/cooldown
sizes came from an event-driven schedule simulation calibrated on
perfetto traces. (Fast DVE reduce paths don't exist: tensor_reduce is
1x by spec and tensor_scalar/scalar_tensor_tensor drop to 1x when an
accumulator output is attached; SWDGE accum-DMA subtraction runs ~4x
below line rate; so sub-on-DVE + reduce-on-ACT is the optimal split.)

Partials land in fp32 columns of a [128, n_tiles] accumulator; the
host combines them in float64 and divides by the global element count.
"""

import numpy as np
import ml_dtypes

import concourse.bacc as bacc
import concourse.mybir as mybir
import concourse.tile as tile
from concourse.bass_utils import run_bass_kernel_spmd

N_CORES = 8
FULL_SHAPE = (64, 128, 4096)
TOTAL_ELEMS = FULL_SHAPE[0] * FULL_SHAPE[1] * FULL_SHAPE[2]  # 33,554,432

P = 128                                   # SBUF partitions
COLS_PER_CORE = TOTAL_ELEMS // N_CORES // P  # 32,768 cols per input tensor

# (dtype, cols, reduce engine) per tile; cols sum to COLS_PER_CORE.
TILES = [
    ("f8", 512, "act"),
    ("f8", 3584, "act"),
    ("f8", 4096, "act"),
    ("f8", 4096, "act"),
    ("f8", 4096, "act"),
    ("f8", 4096, "act"),
    ("b16", 4096, "act"),
    ("b16", 4096, "act"),
    ("b16", 3072, "dve"),
    ("b16", 1024, "act"),
]
assert sum(c for _, c, _ in TILES) == COLS_PER_CORE

_MDT = {"f8": mybir.dt.float8e4, "b16": mybir.dt.bfloat16}
_NDT = {"f8": ml_dtypes.float8_e4m3, "b16": ml_dtypes.bfloat16}

_nc_cache = []


def _build_nc():
    nc = bacc.Bacc("TRN2", target_bir_lowering=False, debug=False)
    zs = [
        nc.declare_dram_parameter(f"z{i}", [P, 2 * c], _MDT[dt], isOutput=False)
        for i, (dt, c, _) in enumerate(TILES)
    ]
    n = len(TILES)
    out = nc.declare_dram_parameter("out", [P, n], mybir.dt.float32, isOutput=True)

    with tile.TileContext(nc) as tc:
        with (
            tc.tile_pool(name="io", bufs=1) as io_pool,
            tc.tile_pool(name="wk", bufs=2) as wk_pool,
            tc.tile_pool(name="acc", bufs=1) as acc_pool,
        ):
            acc = acc_pool.tile([P, n], mybir.dt.float32)
            zts = []
            for i, (dt, c, _) in enumerate(TILES):
                zt = io_pool.tile([P, 2 * c], _MDT[dt], tag=f"z{i}")
                nc.sync.dma_start(zt[:], zs[i][:])
                zts.append(zt)
            for i, (dt, c, eng) in enumerate(TILES):
                d = wk_pool.tile([P, c], mybir.dt.bfloat16, tag="d")
                nc.vector.tensor_sub(d[:], zts[i][:, 0:c], zts[i][:, c : 2 * c])
                if eng == "act":
                    a = wk_pool.tile([P, c], mybir.dt.bfloat16, tag="a")
                    nc.scalar.activation(
                        a[:],
                        d[:],
                        mybir.ActivationFunctionType.Abs,
                        accum_out=acc[:, i : i + 1],
                    )
                else:
                    nc.vector.tensor_reduce(
                        acc[:, i : i + 1],
                        d[:],
                        axis=mybir.AxisListType.X,
                        op=mybir.AluOpType.add,
                        apply_absolute_value=True,
                    )
            nc.sync.dma_start(out[:], acc[:])
    nc.compile()
    return nc


def _get_nc():
    if not _nc_cache:
        _nc_cache.append(_build_nc())
    return _nc_cache[0]


def _shard_inputs(yhat: np.ndarray, y: np.ndarray) -> list[dict[str, np.ndarray]]:
    yhat_m = np.ascontiguousarray(yhat, dtype=np.float32).reshape(
        N_CORES, P, COLS_PER_CORE
    )
    y_m = np.ascontiguousarray(y, dtype=np.float32).reshape(
        N_CORES, P, COLS_PER_CORE
    )
    in_maps = [{} for _ in range(N_CORES)]
    off = 0
    for i, (dt, c, _) in enumerate(TILES):
        zt = np.empty((N_CORES, P, 2 * c), dtype=_NDT[dt])
        zt[:, :, 0:c] = yhat_m[:, :, off : off + c]
        zt[:, :, c : 2 * c] = y_m[:, :, off : off + c]
        for core in range(N_CORES):
            in_maps[core][f"z{i}"] = zt[core]
        off += c
    return in_maps


def kernel(yhat: np.ndarray, y: np.ndarray) -> np.ndarray:
    nc = _get_nc()
    in_maps = _shard_inputs(yhat, y)
    res = run_bass_kernel_spmd(nc, in_maps, list(range(N_CORES)))
    total = np.float64(0.0)
    for r in res.results:
        total += r["out"].astype(np.float64).sum()
    return np.asarray(total / TOTAL_ELEMS, dtype=np.float32)
